# revision 4
# baseline (speedup 1.0000x reference)
"""Trainium2 Bass kernel for a Bahdanau attention decoder step (B=1).

Shapes (hardcoded): H=1024, V=50257, L=2048, B=1, 8 NeuronCores.

Sharding:
  - vocab dim padded to 51200 and sharded 6400/core for the output
    projection (out_W staged pre-transposed per core: [2H, 6400]).
  - embedding column-sharded [V, 128] per core; the looked-up row slice is
    AllGather'ed into the full embedded vector `we`.
  - GRU sharded by gate-slice: core k computes rows k*128:(k+1)*128 of each
    of the r/z/n gates and its h_new slice; h_new is AllGather'ed.
  - attention replicated (encoder_outputs + attn_W[:, H:] on every core).

Collectives: AllGather(we) at t~0 (overlapped), AllGather(h_new),
AllGather(logsumexp stats). log_softmax = logits - (M + log sum_j S_j e^{m_j-M}).
"""
import sys, os, ctypes, contextlib

if "/opt/trn_rl_repo" not in sys.path:
    sys.path.insert(0, "/opt/trn_rl_repo")

import numpy as np

H = 1024
V = 50257
L = 2048
NC = 8
VP = 51200          # padded vocab
VS = VP // NC       # 6400 per core
HC = H // 128       # 8 hidden chunks
LT = L // 128       # 16 encoder tiles
VB = 1024           # logits vocab block width
DEBUG = bool(int(os.environ.get("NN_KERNEL_DEBUG", "0")))

_CACHE = {}


def _vblocks():
    out = []
    off = 0
    while off < VS:
        wid = min(VB, VS - off)
        out.append((off, wid))
        off += wid
    return out


def _build():
    import concourse.bass as bass
    import concourse.bacc as bacc
    import concourse.mybir as mybir
    import concourse.tile as tile

    f32 = mybir.dt.float32
    i32 = mybir.dt.int32
    AF = mybir.ActivationFunctionType
    ALU = mybir.AluOpType
    AX = mybir.AxisListType

    nc = bacc.Bacc("TRN2", target_bir_lowering=False, debug=False, num_devices=NC)

    # ---- inputs (per-core shards staged by host) ----
    idx2 = nc.dram_tensor("idx2", [2, 1], i32, kind="ExternalInput")
    emb_cs = nc.dram_tensor("emb_cs", [V, 128], f32, kind="ExternalInput")
    enc = nc.dram_tensor("enc", [L, H], f32, kind="ExternalInput")
    w2 = nc.dram_tensor("w2", [H, H], f32, kind="ExternalInput")
    vvec = nc.dram_tensor("vvec", [1, H], f32, kind="ExternalInput")
    wih = nc.dram_tensor("wih", [384, 2 * H], f32, kind="ExternalInput")
    whh = nc.dram_tensor("whh", [384, H], f32, kind="ExternalInput")
    bih = nc.dram_tensor("bih", [3, 128], f32, kind="ExternalInput")
    bhh = nc.dram_tensor("bhh", [3, 128], f32, kind="ExternalInput")
    hfull = nc.dram_tensor("hfull", [1, H], f32, kind="ExternalInput")
    hsh = nc.dram_tensor("hsh", [128, 1], f32, kind="ExternalInput")
    wot = nc.dram_tensor("wot", [2 * H, VS], f32, kind="ExternalInput")
    outb = nc.dram_tensor("outb", [1, VS], f32, kind="ExternalInput")
    ident_in = nc.dram_tensor("ident_in", [128, 128], f32, kind="ExternalInput")

    # ---- outputs ----
    o_logp = nc.dram_tensor("o_logp", [1, VS], f32, kind="ExternalOutput")
    o_hnew = nc.dram_tensor("o_hnew", [128, 1], f32, kind="ExternalOutput")
    o_attnw = nc.dram_tensor("o_attnw", [LT, 128], f32, kind="ExternalOutput")
    if DEBUG:
        d_we = nc.dram_tensor("d_we", [1, H], f32, kind="ExternalOutput")
        d_u2 = nc.dram_tensor("d_u2", [1, H], f32, kind="ExternalOutput")
        d_scores = nc.dram_tensor("d_scores", [LT, 128], f32, kind="ExternalOutput")
        d_ctx = nc.dram_tensor("d_ctx", [1, H], f32, kind="ExternalOutput")
        d_gi = nc.dram_tensor("d_gi", [3, 128], f32, kind="ExternalOutput")
        d_gh = nc.dram_tensor("d_gh", [3, 128], f32, kind="ExternalOutput")
        d_x2c = nc.dram_tensor("d_x2c", [2, 2 * HC], f32, kind="ExternalOutput")
        d_lg = nc.dram_tensor("d_lg", [1, VS], f32, kind="ExternalOutput")

    RG = [list(range(NC))]

    with tile.TileContext(nc) as tc:
        with tc.tile_pool(name="dram", bufs=1, space="DRAM") as dram, \
             tc.tile_pool(name="cst", bufs=1) as cst, \
             tc.tile_pool(name="enc_p", bufs=3) as enc_p, \
             tc.tile_pool(name="wo_p", bufs=4) as wo_p, \
             tc.tile_pool(name="w2_p", bufs=2) as w2_p, \
             tc.tile_pool(name="g_p", bufs=2) as g_p, \
             tc.tile_pool(name="scr", bufs=2) as scr_p, \
             tc.tile_pool(name="row_p", bufs=1) as row_p, \
             tc.tile_pool(name="rowc_p", bufs=2) as rowc_p, \
             tc.tile_pool(name="ps_acc", bufs=2, space="PSUM") as ps_acc, \
             tc.tile_pool(name="ps_t", bufs=2, space="PSUM") as ps_t:

            def t_ps(name):
                return ps_t.tile([128, 128], f32, tag="tps", name=name)

            ident = cst.tile([128, 128], f32)
            nc.sync.dma_start(ident[:], ident_in[:])

            def part_bcast(src11, name):
                """[1,1] sbuf scalar -> [128,1] sbuf replicated (PE transpose)."""
                p = t_ps("ps_" + name)
                nc.tensor.transpose(out=p[:, 0:1], in_=src11[:].to_broadcast([1, 128]),
                                    identity=ident[0:1, 0:1])
                o = cst.tile([128, 1], f32, name=name)
                nc.vector.tensor_copy(o[:], p[:, 0:1])
                return o

            def part_reduce(col, op, name):
                """[128,1] sbuf -> [1,1] sbuf reduced across partitions."""
                p = t_ps("ps_" + name)
                nc.tensor.transpose(out=p[0:1, :], in_=col[:], identity=ident[:])
                row = cst.tile([1, 128], f32, name=name + "_row")
                nc.vector.tensor_copy(row[:], p[0:1, :])
                o = cst.tile([1, 1], f32, name=name)
                nc.vector.tensor_reduce(o[:], row[:], axis=AX.X, op=op)
                return o

            # ---- 1. embedding gather + AllGather(we) (fires immediately) ----
            idx_sb = cst.tile([2, 1], i32)
            nc.sync.dma_start(idx_sb[:], idx2[:])
            we_g = cst.tile([2, 128], f32)
            nc.gpsimd.indirect_dma_start(
                out=we_g[:], out_offset=None, in_=emb_cs[:],
                in_offset=bass.IndirectOffsetOnAxis(ap=idx_sb[:, :1], axis=0))
            cwe_in = dram.tile([1, 128], f32)
            cwe_out = dram.tile([1, H], f32, addr_space="Shared")
            nc.sync.dma_start(cwe_in[:], we_g[0:1, :])
            nc.gpsimd.collective_compute(
                "AllGather", mybir.AluOpType.bypass, replica_groups=RG,
                ins=[cwe_in[:].opt()], outs=[cwe_out[:].opt()])

            # ---- 2. u2 = v @ W2 (replicated)  [1, H] on partition 0 ----
            v_c = cst.tile([128, HC], f32)
            nc.sync.dma_start(v_c[:], vvec[0:1, :].rearrange("a (c p) -> (a p) c", p=128))
            ps_u = ps_acc.tile([1, VB], f32, tag="acc", name="ps_u", space="PSUM")
            for hc in range(HC):
                w2_t = w2_p.tile([128, H], f32, tag="w2t", name=f"w2_{hc}")
                nc.sync.dma_start(w2_t[:], w2[hc * 128:(hc + 1) * 128, :])
                for j in range(2):
                    nc.tensor.matmul(ps_u[0:1, j * 512:(j + 1) * 512],
                                     lhsT=v_c[:, hc:hc + 1],
                                     rhs=w2_t[:, j * 512:(j + 1) * 512],
                                     start=(hc == 0), stop=(hc == HC - 1),
                                     skip_group_check=True)
            u2_row = cst.tile([1, H], f32)
            nc.vector.tensor_copy(u2_row[:], ps_u[0:1, 0:H])
            u2_dram = dram.tile([1, H], f32)
            nc.sync.dma_start(u2_dram[:], u2_row[:])
            u2_rep = cst.tile([128, H], f32)
            nc.sync.dma_start(u2_rep[:], u2_dram[:].to_broadcast([128, H]))
            if DEBUG:
                nc.sync.dma_start(d_u2[:], u2_row[:])

            # ---- 3. scores = enc @ u2  (DVE) ----
            scores = cst.tile([128, LT], f32)
            for t in range(LT):
                et = enc_p.tile([128, H], f32, tag="enc", name=f"enc_{t}")
                nc.sync.dma_start(et[:], enc[t * 128:(t + 1) * 128, :])
                sc = scr_p.tile([128, 2 * H], f32, tag="scr", name=f"sscr_{t}")
                nc.vector.tensor_tensor(out=sc[:, 0:H], in0=et[:], in1=u2_rep[:],
                                        op=ALU.mult)
                nc.vector.tensor_reduce(scores[:, t:t + 1], sc[:, 0:H], axis=AX.X,
                                        op=ALU.add)
            if DEBUG:
                nc.sync.dma_start(d_scores[:].rearrange("t p -> p t"), scores[:])

            # ---- 4. softmax(scores) (replicated) ----
            smax_c = cst.tile([128, 1], f32)
            nc.vector.tensor_reduce(smax_c[:], scores[:], axis=AX.X, op=ALU.max)
            sm = part_reduce(smax_c, ALU.max, "sm")
            nsm = cst.tile([1, 1], f32)
            nc.vector.tensor_scalar_mul(nsm[:], sm[:], -1.0)
            nsm_bc = part_bcast(nsm, "nsm_bc")
            attw_e = cst.tile([128, LT], f32)
            esum_c = cst.tile([128, 1], f32)
            nc.scalar.activation(attw_e[:], scores[:], AF.Exp,
                                 bias=nsm_bc[:], scale=1.0, accum_out=esum_c[:])
            zsum = part_reduce(esum_c, ALU.add, "zsum")
            rz = cst.tile([1, 1], f32)
            nc.vector.reciprocal(rz[:], zsum[:])
            rz_bc = part_bcast(rz, "rz_bc")
            attw = cst.tile([128, LT], f32)
            nc.scalar.mul(attw[:], attw_e[:], rz_bc[:])
            nc.sync.dma_start(o_attnw[:].rearrange("t p -> p t"), attw[:])

            # ---- 5. context = attw @ enc (PE over l, enc re-streamed) ----
            ps_c = ps_acc.tile([1, VB], f32, tag="acc", name="ps_c", space="PSUM")
            for t in range(LT):
                et = enc_p.tile([128, H], f32, tag="enc", name=f"enc2_{t}")
                nc.sync.dma_start(et[:], enc[t * 128:(t + 1) * 128, :])
                for j in range(2):
                    nc.tensor.matmul(ps_c[0:1, j * 512:(j + 1) * 512],
                                     lhsT=attw[:, t:t + 1],
                                     rhs=et[:, j * 512:(j + 1) * 512],
                                     start=(t == 0), stop=(t == LT - 1),
                                     skip_group_check=True)
            ctx_row = cst.tile([1, H], f32)
            nc.vector.tensor_copy(ctx_row[:], ps_c[0:1, 0:H])
            ctx_dram = dram.tile([1, H], f32)
            nc.sync.dma_start(ctx_dram[:], ctx_row[:])
            if DEBUG:
                nc.sync.dma_start(d_ctx[:], ctx_row[:])

            # ---- 6. GRU slice-k ----
            h_rep = cst.tile([128, H], f32)
            nc.sync.dma_start(h_rep[:], hfull[0:1, :].to_broadcast([128, H]))
            ghcol = cst.tile([128, 3], f32)
            for g in range(3):
                wt = g_p.tile([128, H], f32, tag="whh", name=f"whh_{g}")
                nc.sync.dma_start(wt[:], whh[g * 128:(g + 1) * 128, :])
                sc = scr_p.tile([128, 2 * H], f32, tag="scr", name=f"ghscr_{g}")
                nc.vector.tensor_tensor(out=sc[:, 0:H], in0=wt[:], in1=h_rep[:],
                                        op=ALU.mult)
                nc.vector.tensor_reduce(ghcol[:, g:g + 1], sc[:, 0:H], axis=AX.X,
                                        op=ALU.add)

            x_rep = cst.tile([128, 2 * H], f32)
            nc.sync.dma_start(x_rep[:, 0:H], cwe_out[:].to_broadcast([128, H]))
            nc.sync.dma_start(x_rep[:, H:2 * H], ctx_dram[:].to_broadcast([128, H]))
            if DEBUG:
                nc.sync.dma_start(d_we[:], x_rep[0:1, 0:H])
            gicol = cst.tile([128, 3], f32)
            for g in range(3):
                wt = g_p.tile([128, 2 * H], f32, tag="wih", name=f"wih_{g}")
                nc.sync.dma_start(wt[:], wih[g * 128:(g + 1) * 128, :])
                sc = scr_p.tile([128, 2 * H], f32, tag="scr", name=f"giscr_{g}")
                nc.vector.tensor_tensor(out=sc[:], in0=wt[:], in1=x_rep[:],
                                        op=ALU.mult)
                nc.vector.tensor_reduce(gicol[:, g:g + 1], sc[:], axis=AX.X,
                                        op=ALU.add)
            if DEBUG:
                nc.sync.dma_start(d_gi[:].rearrange("g p -> p g"), gicol[:])
                nc.sync.dma_start(d_gh[:].rearrange("g p -> p g"), ghcol[:])

            bih_sb = cst.tile([128, 3], f32)
            nc.sync.dma_start(bih_sb[:], bih[:].rearrange("g p -> p g"))
            bhh_sb = cst.tile([128, 3], f32)
            nc.sync.dma_start(bhh_sb[:], bhh[:].rearrange("g p -> p g"))
            gib = cst.tile([128, 3], f32)
            nc.vector.tensor_tensor(out=gib[:], in0=gicol[:], in1=bih_sb[:], op=ALU.add)
            ghb = cst.tile([128, 3], f32)
            nc.vector.tensor_tensor(out=ghb[:], in0=ghcol[:], in1=bhh_sb[:], op=ALU.add)

            rzpre = cst.tile([128, 2], f32)
            nc.vector.tensor_tensor(out=rzpre[:], in0=gib[:, 0:2], in1=ghb[:, 0:2],
                                    op=ALU.add)
            rzg = cst.tile([128, 2], f32)
            nc.scalar.activation(rzg[:], rzpre[:], AF.Sigmoid)
            npre = cst.tile([128, 1], f32)
            nc.vector.tensor_tensor(out=npre[:], in0=rzg[:, 0:1], in1=ghb[:, 2:3],
                                    op=ALU.mult)
            nc.vector.tensor_tensor(out=npre[:], in0=npre[:], in1=gib[:, 2:3],
                                    op=ALU.add)
            ngate = cst.tile([128, 1], f32)
            nc.scalar.activation(ngate[:], npre[:], AF.Tanh)
            hsh_sb = cst.tile([128, 1], f32)
            nc.sync.dma_start(hsh_sb[:], hsh[:])
            hmn = cst.tile([128, 1], f32)
            nc.vector.tensor_tensor(out=hmn[:], in0=hsh_sb[:], in1=ngate[:],
                                    op=ALU.subtract)
            zh = cst.tile([128, 1], f32)
            nc.vector.tensor_tensor(out=zh[:], in0=rzg[:, 1:2], in1=hmn[:],
                                    op=ALU.mult)
            hnew = cst.tile([128, 1], f32)
            nc.vector.tensor_tensor(out=hnew[:], in0=ngate[:], in1=zh[:], op=ALU.add)
            nc.sync.dma_start(o_hnew[:], hnew[:])

            # ---- 7. AllGather(h_new) ----
            c4_in = dram.tile([1, 128], f32)
            nc.sync.dma_start(c4_in[0:1, :].rearrange("a b -> b a"), hnew[:])
            c4_out = dram.tile([1, H], f32, addr_space="Shared")
            nc.gpsimd.collective_compute(
                "AllGather", mybir.AluOpType.bypass, replica_groups=RG,
                ins=[c4_in[:].opt()], outs=[c4_out[:].opt()])

            # ---- 8. logits = [h_new; ctx] @ out_W_shard.T (PE, two passes) ----
            x2c = cst.tile([128, 2 * HC], f32)
            nc.sync.dma_start(x2c[:, 0:HC],
                              c4_out[0:1, :].rearrange("a (c p) -> (a p) c", p=128))
            nc.sync.dma_start(x2c[:, HC:2 * HC],
                              ctx_dram[0:1, :].rearrange("a (c p) -> (a p) c", p=128))
            if DEBUG:
                nc.sync.dma_start(d_x2c[0:1, :], x2c[0:1, :])
                nc.sync.dma_start(d_x2c[1:2, :], x2c[96:97, :])

            lg_row = row_p.tile([1, VS], f32, tag="lg", name="lg_row")
            vbs = _vblocks()
            # pass 1: context half (k-chunks HC..2*HC-1 of x2)
            for off, wid in vbs:
                ps_l = ps_acc.tile([1, VB], f32, tag="acc", name=f"psl_c{off}",
                                   space="PSUM")
                for i, k in enumerate(range(HC, 2 * HC)):
                    wo_t = wo_p.tile([128, VB], f32, tag="wot", name=f"wo_c{off}_{k}")
                    nc.sync.dma_start(wo_t[:, 0:wid],
                                      wot[k * 128:(k + 1) * 128, off:off + wid])
                    for j in range(0, wid, 512):
                        nj = min(512, wid - j)
                        nc.tensor.matmul(ps_l[0:1, j:j + nj],
                                         lhsT=x2c[:, k:k + 1],
                                         rhs=wo_t[:, j:j + nj],
                                         start=(i == 0), stop=(i == HC - 1),
                                         skip_group_check=True)
                nc.vector.tensor_copy(lg_row[0:1, off:off + wid], ps_l[0:1, 0:wid])
            # pass 2: h_new half (k-chunks 0..HC-1), add onto pass-1 partials
            for off, wid in vbs:
                ps_l = ps_acc.tile([1, VB], f32, tag="acc", name=f"psl_h{off}",
                                   space="PSUM")
                for i, k in enumerate(range(HC)):
                    wo_t = wo_p.tile([128, VB], f32, tag="wot", name=f"wo_h{off}_{k}")
                    nc.sync.dma_start(wo_t[:, 0:wid],
                                      wot[k * 128:(k + 1) * 128, off:off + wid])
                    for j in range(0, wid, 512):
                        nj = min(512, wid - j)
                        nc.tensor.matmul(ps_l[0:1, j:j + nj],
                                         lhsT=x2c[:, k:k + 1],
                                         rhs=wo_t[:, j:j + nj],
                                         start=(i == 0), stop=(i == HC - 1),
                                         skip_group_check=True)
                nc.vector.tensor_tensor(out=lg_row[0:1, off:off + wid],
                                        in0=lg_row[0:1, off:off + wid],
                                        in1=ps_l[0:1, 0:wid], op=ALU.add)
                ob_c = rowc_p.tile([1, VB], f32, tag="obc", name=f"ob_{off}")
                nc.sync.dma_start(ob_c[0:1, 0:wid], outb[0:1, off:off + wid])
                nc.vector.tensor_tensor(out=lg_row[0:1, off:off + wid],
                                        in0=lg_row[0:1, off:off + wid],
                                        in1=ob_c[0:1, 0:wid], op=ALU.add)
            if DEBUG:
                nc.sync.dma_start(d_lg[:], lg_row[:])

            # ---- 9. local logsumexp stats + AllGather (single partition) ----
            lm = cst.tile([1, 1], f32)
            nc.vector.tensor_reduce(lm[:], lg_row[:], axis=AX.X, op=ALU.max)
            nlm = cst.tile([1, 1], f32)
            nc.vector.tensor_scalar_mul(nlm[:], lm[:], -1.0)
            lex = row_p.tile([1, VS], f32, tag="lex", name="lex")
            ls = cst.tile([1, 1], f32)
            nc.scalar.activation(lex[:], lg_row[:], AF.Exp,
                                 bias=nlm[:], scale=1.0, accum_out=ls[:])
            st_sb = cst.tile([1, 2], f32)
            nc.vector.tensor_copy(st_sb[:, 0:1], lm[:])
            nc.vector.tensor_copy(st_sb[:, 1:2], ls[:])
            st_in = dram.tile([1, 64], f32)
            nc.sync.dma_start(st_in[0:1, 0:2], st_sb[:])
            st_out = dram.tile([1, 64 * NC], f32, addr_space="Shared")
            nc.gpsimd.collective_compute(
                "AllGather", mybir.AluOpType.bypass, replica_groups=RG,
                ins=[st_in[:].opt()], outs=[st_out[:].opt()])

            # ---- 10. global lse, final log-probs ----
            st_a = cst.tile([1, 64 * NC], f32)
            nc.sync.dma_start(st_a[:], st_out[:])
            st_v = st_a[:].rearrange("a (j r) -> a r j", r=64)   # [1, 64, 8]
            mvals = cst.tile([1, NC], f32)
            nc.vector.tensor_copy(mvals[:], st_v[:, 0:1, :])
            svals = cst.tile([1, NC], f32)
            nc.vector.tensor_copy(svals[:], st_v[:, 1:2, :])
            gM = cst.tile([1, 1], f32)
            nc.vector.tensor_reduce(gM[:], mvals[:], axis=AX.X, op=ALU.max)
            ngM = cst.tile([1, 1], f32)
            nc.vector.tensor_scalar_mul(ngM[:], gM[:], -1.0)
            dvals = cst.tile([1, NC], f32)
            nc.scalar.activation(dvals[:], mvals[:], AF.Exp, bias=ngM[:], scale=1.0)
            tvals = cst.tile([1, NC], f32)
            nc.vector.tensor_tensor(out=tvals[:], in0=dvals[:], in1=svals[:],
                                    op=ALU.mult)
            gZ = cst.tile([1, 1], f32)
            nc.vector.tensor_reduce(gZ[:], tvals[:], axis=AX.X, op=ALU.add)
            lnZ = cst.tile([1, 1], f32)
            nc.scalar.activation(lnZ[:], gZ[:], AF.Ln)
            lse = cst.tile([1, 1], f32)
            nc.vector.tensor_tensor(out=lse[:], in0=lnZ[:], in1=gM[:], op=ALU.add)
            nlse = cst.tile([1, 1], f32)
            nc.vector.tensor_scalar_mul(nlse[:], lse[:], -1.0)
            logp = row_p.tile([1, VS], f32, tag="lex", name="logp")
            nc.scalar.activation(logp[:], lg_row[:], AF.Identity,
                                 bias=nlse[:], scale=1.0)
            nc.sync.dma_start(o_logp[:], logp[:])

    nc.compile()
    return nc


def _prep_in_maps(inputs):
    wi = np.asarray(inputs["word_input"]).astype(np.int64).reshape(-1)
    emb = np.asarray(inputs["emb"], dtype=np.float32)
    enc = np.asarray(inputs["encoder_outputs"], dtype=np.float32).reshape(L, H)
    attn_W = np.asarray(inputs["attn_W"], dtype=np.float32)
    v = np.asarray(inputs["v"], dtype=np.float32).reshape(1, H)
    W_ih = np.asarray(inputs["W_ih"], dtype=np.float32)
    W_hh = np.asarray(inputs["W_hh"], dtype=np.float32)
    b_ih = np.asarray(inputs["b_ih"], dtype=np.float32).reshape(-1)
    b_hh = np.asarray(inputs["b_hh"], dtype=np.float32).reshape(-1)
    h = np.asarray(inputs["last_hidden"], dtype=np.float32).reshape(1, H)
    out_W = np.asarray(inputs["out_W"], dtype=np.float32)
    out_b = np.asarray(inputs["out_b"], dtype=np.float32).reshape(-1)

    idx2 = np.full((2, 1), int(wi[0]), dtype=np.int32)
    w2 = np.ascontiguousarray(attn_W[:, H:2 * H])
    ident = np.eye(128, dtype=np.float32)

    Wpad = np.zeros((VP, 2 * H), dtype=np.float32)
    Wpad[:V] = out_W
    bpad = np.full((VP,), -1e30, dtype=np.float32)
    bpad[:V] = out_b

    in_maps = []
    for k in range(NC):
        rows = np.concatenate([np.arange(g * H + k * 128, g * H + (k + 1) * 128)
                               for g in range(3)])
        in_maps.append({
            "idx2": idx2,
            "emb_cs": np.ascontiguousarray(emb[:, k * 128:(k + 1) * 128]),
            "enc": enc,
            "w2": w2,
            "vvec": v,
            "wih": np.ascontiguousarray(W_ih[rows]),
            "whh": np.ascontiguousarray(W_hh[rows]),
            "bih": np.ascontiguousarray(b_ih[rows].reshape(3, 128)),
            "bhh": np.ascontiguousarray(b_hh[rows].reshape(3, 128)),
            "hfull": h,
            "hsh": np.ascontiguousarray(h[0, k * 128:(k + 1) * 128].reshape(128, 1)),
            "wot": np.ascontiguousarray(Wpad[k * VS:(k + 1) * VS].T),
            "outb": np.ascontiguousarray(bpad[k * VS:(k + 1) * VS].reshape(1, VS)),
            "ident_in": ident,
        })
    return in_maps


@contextlib.contextmanager
def _maybe_profile():
    prof_dir = os.environ.get("NN_PROF_DIR")
    if not prof_dir:
        yield
        return
    import jax
    jax.devices()
    lib = ctypes.CDLL("/opt/axon/libaxon_pjrt.so")
    lib.axon_start_nrt_profile.argtypes = [ctypes.POINTER(ctypes.c_int64),
                                           ctypes.c_size_t]
    lib.axon_start_nrt_profile.restype = ctypes.c_int64
    lib.axon_stop_nrt_profile.argtypes = [ctypes.c_char_p]
    lib.axon_stop_nrt_profile.restype = ctypes.c_int64
    ids = (ctypes.c_int64 * 1)(0)
    rc = lib.axon_start_nrt_profile(ids, 1)
    if rc != 0:
        raise RuntimeError(f"axon_start_nrt_profile rc={rc}")
    try:
        yield
    finally:
        n = lib.axon_stop_nrt_profile(str(prof_dir).encode())
        print(f"profile: {n} file(s) written to {prof_dir}")


def kernel(**inputs):
    from concourse import bass_utils

    if "nc" not in _CACHE:
        _CACHE["nc"] = _build()
    nc = _CACHE["nc"]
    in_maps = _prep_in_maps(inputs)
    with _maybe_profile():
        res = bass_utils.run_bass_kernel_spmd(nc, in_maps, core_ids=list(range(NC)))

    if DEBUG:
        _CACHE["last_results"] = res.results

    logp = np.concatenate([res.results[k]["o_logp"].reshape(VS) for k in range(NC)])
    log_probs = logp[:V].reshape(1, V)
    h_new = np.concatenate([res.results[k]["o_hnew"].reshape(128)
                            for k in range(NC)]).reshape(1, 1, H)
    attn_w = res.results[0]["o_attnw"].reshape(L).reshape(1, 1, L)
    return log_probs, h_new, attn_w
